# revision 3
# baseline (speedup 1.0000x reference)
# Cost-volume concatenation kernel for Trainium2 (Bass/Tile), SPMD over 8 cores.
#
# Problem: left, right: [B=2, H=64, W=256, C=32] f32.
# out[b, d+48, h, w, :32] = left[b,h,w,:]  * valid(w,d)
# out[b, d+48, h, w, 32:] = right[b,h,w-d,:] * valid(w,d),  d in [-48, 48)
# valid(w,d) = 0 <= w-d < W.  Output [2, 96, 64, 256, 64] f32 (~805 MB).
#
# Sharding: disparity axis. Core k handles the 12 levels d in [12k-48, 12k-36).
# The kernel program is identical on every core; all per-core variation lives in
# the DATA:
#   - rpad:  right pre-shifted by the core's base disparity and zero-padded to
#            width TPAD, so the in-kernel shift is j in [0,12) for every core and
#            the zero padding implements the right-half validity masking.
#   - vrep:  a 0/1 validity mask with the same index structure, replicated
#            across the 128 SBUF partitions; out_left = left * vrep_shifted
#            implements the left-half masking.
# SBUF layout: partitions = (b, h) = 2*64 = 128; free dim = (w, c).
# Per disparity j the kernel assembles interleaved [left|right] rows in SBUF
# (one f32 tensor_mul + one tensor_copy per w-chunk) and streams them to HBM
# with large contiguous DMAs. Per-core traffic: ~13 MB read + ~100 MB write.

import numpy as np

B, H, W, C = 2, 64, 256, 32
MAX_DISP = 48
D2 = 2 * MAX_DISP            # 96 disparity levels
N_CORES = 8
DPC = D2 // N_CORES          # 12 disparities per core
JPAD = DPC - 1               # 11: shift offset so in-kernel shifts are >= 0
TPAD = 272                   # padded t-width (>= W + JPAD = 267)
P = B * H                    # 128 SBUF partitions = (b, h)
WC = W * C                   # 8192
TC = TPAD * C                # 8704
WCHUNK = 128                 # w-columns per output tile / DMA (4 MB per DMA)
F32 = np.float32

_CACHE = {}


def _build_nc():
    import concourse.bacc as bacc
    import concourse.mybir as mybir
    from concourse.tile import TileContext

    f32 = mybir.dt.float32
    nc = bacc.Bacc("TRN2", target_bir_lowering=False, debug=False)
    left_t = nc.dram_tensor("left_flat", [P, WC], f32, kind="ExternalInput")
    rpad_t = nc.dram_tensor("rpad", [P, TC], f32, kind="ExternalInput")
    vrep_t = nc.dram_tensor("vrep", [P, TC], f32, kind="ExternalInput")
    out_t = nc.dram_tensor("out", [B, DPC, H, W, 2 * C], f32, kind="ExternalOutput")

    with TileContext(nc) as tc:
        with (
            tc.tile_pool(name="ins", bufs=1) as ipool,
            tc.tile_pool(name="outs", bufs=2) as opool,
        ):
            left_sb = ipool.tile([P, WC], f32, tag="left")
            rpad_sb = ipool.tile([P, TC], f32, tag="rpad")
            vrep_sb = ipool.tile([P, TC], f32, tag="vrep")
            nc.sync.dma_start(out=left_sb[:], in_=left_t[:])
            nc.sync.dma_start(out=rpad_sb[:], in_=rpad_t[:])
            nc.sync.dma_start(out=vrep_sb[:], in_=vrep_t[:])

            lv = left_sb[:].rearrange("p (w c) -> p w c", c=C)
            rv = rpad_sb[:].rearrange("p (t c) -> p t c", c=C)
            vv = vrep_sb[:].rearrange("p (t c) -> p t c", c=C)

            for j in range(DPC):
                for wi in range(0, W, WCHUNK):
                    ot = opool.tile([P, WCHUNK * 2 * C], f32, tag="ot")
                    ov = ot[:].rearrange("p (w c) -> p w c", c=2 * C)
                    t0 = wi + JPAD - j
                    nc.vector.tensor_mul(
                        out=ov[:, :, 0:C],
                        in0=lv[:, wi : wi + WCHUNK, :],
                        in1=vv[:, t0 : t0 + WCHUNK, :],
                    )
                    # tensor_mul (not tensor_copy): f32 SBUF tensor_copy runs
                    # DVE in 2-port perf mode, which locks GpSimd out of the
                    # shared SBUF port pair and starves SWDGE descriptor
                    # generation (the output DMAs). TT ops stay in 1x mode.
                    # rpad is already zero outside the valid region, so
                    # multiplying by vrep is the identity there.
                    nc.vector.tensor_mul(
                        out=ov[:, :, C : 2 * C],
                        in0=rv[:, t0 : t0 + WCHUNK, :],
                        in1=vv[:, t0 : t0 + WCHUNK, :],
                    )
                    nc.gpsimd.dma_start(
                        out=out_t[:, j, :, wi : wi + WCHUNK, :],
                        in_=ot[:],
                    )
    nc.finalize()
    return nc


def get_nc():
    if "nc" not in _CACHE:
        _CACHE["nc"] = _build_nc()
    return _CACHE["nc"]


def prep_inputs(left, right):
    """Build the 8 per-core input maps from full left/right."""
    left = np.ascontiguousarray(left, dtype=F32)
    right = np.ascontiguousarray(right, dtype=F32)
    left_flat = left.reshape(P, WC)
    in_maps = []
    for k in range(N_CORES):
        d0 = DPC * k - MAX_DISP
        shift = JPAD + d0        # rpad[..., t, :] = right[..., t - shift, :]
        rpad = np.zeros((B, H, TPAD, C), F32)
        lo, hi = max(0, shift), min(TPAD, shift + W)
        if lo < hi:
            rpad[:, :, lo:hi, :] = right[:, :, lo - shift : hi - shift, :]
        vk = np.zeros(TPAD, F32)
        vk[lo:hi] = 1.0
        vrep = np.ascontiguousarray(
            np.broadcast_to(np.repeat(vk, C), (P, TC)), dtype=F32
        )
        in_maps.append(
            {"left_flat": left_flat, "rpad": rpad.reshape(P, TC), "vrep": vrep}
        )
    return in_maps


def run(left, right, **kwargs):
    """Run the SPMD kernel; returns (full_output, BassKernelResults)."""
    from concourse.bass_utils import run_bass_kernel_spmd

    nc = get_nc()
    in_maps = prep_inputs(left, right)
    res = run_bass_kernel_spmd(nc, in_maps, core_ids=list(range(N_CORES)), **kwargs)
    full = np.concatenate([r["out"] for r in res.results], axis=1)
    return full, res


def kernel(left, right):
    full, _ = run(left, right)
    return full


# revision 4
# speedup vs baseline: 4.0073x; 4.0073x over previous
# Cost-volume concatenation kernel for Trainium2 (Bass/Tile), SPMD over 8 cores.
#
# Problem: left, right: [B=2, H=64, W=256, C=32] f32.
# out[b, d+48, h, w, :32] = left[b,h,w,:]  * valid(w,d)
# out[b, d+48, h, w, 32:] = right[b,h,w-d,:] * valid(w,d),  d in [-48, 48)
# valid(w,d) = 0 <= w-d < W.  Output [2, 96, 64, 256, 64] f32 (~805 MB).
#
# Sharding: disparity axis. Core k handles the 12 levels d in [12k-48, 12k-36).
# The kernel program is identical on every core; all per-core variation lives in
# the DATA:
#   - rpad:  right pre-shifted by the core's base disparity and zero-padded to
#            width TPAD, so the in-kernel shift is j in [0,12) for every core and
#            the zero padding implements the right-half validity masking.
#   - vrep:  a 0/1 validity mask with the same index structure, replicated
#            across the 128 SBUF partitions; out_left = left * vrep_shifted
#            implements the left-half masking.
#
# SBUF layout: partitions = (h, b) — h-major — p = 2*h + b, 128 partitions;
# free dim = (w, c). h-major matters: the output DMA's DRAM access pattern is
# then [h=64, b=2, wc] with outer dim 64, which HWDGE fans out across all 16
# SDMA engines. (A b-major [2, 64, wc] pattern splits over only 2 engines ->
# ~27 GB/s per core; SWDGE spreads by partition but its descriptor ring
# backpressure caps concurrency at ~4 engines for multi-descriptor transfers.)
#
# Per disparity j the kernel assembles interleaved [left|right] rows in SBUF
# (two f32 tensor ops per w-chunk) and streams them out with 4 MB contiguous
# HWDGE DMAs. Per-core traffic: ~13 MB read + ~100 MB write (memory-bound).

import numpy as np

B, H, W, C = 2, 64, 256, 32
MAX_DISP = 48
D2 = 2 * MAX_DISP            # 96 disparity levels
N_CORES = 8
DPC = D2 // N_CORES          # 12 disparities per core
JPAD = DPC - 1               # 11: shift offset so in-kernel shifts are >= 0
TPAD = 272                   # padded t-width (>= W + JPAD = 267)
P = B * H                    # 128 SBUF partitions = (h, b) h-major
WC = W * C                   # 8192
TC = TPAD * C                # 8704
WCHUNK = 128                 # w-columns per output tile / DMA (4 MB per DMA)
F32 = np.float32

_CACHE = {}


def _build_nc():
    import concourse.bacc as bacc
    import concourse.mybir as mybir
    from concourse.tile import TileContext

    f32 = mybir.dt.float32
    nc = bacc.Bacc("TRN2", target_bir_lowering=False, debug=False)
    left_t = nc.dram_tensor("left_flat", [P, WC], f32, kind="ExternalInput")
    rpad_t = nc.dram_tensor("rpad", [P, TC], f32, kind="ExternalInput")
    vrep_t = nc.dram_tensor("vrep", [P, TC], f32, kind="ExternalInput")
    out_t = nc.dram_tensor("out", [B, DPC, H, W * 2 * C], f32, kind="ExternalOutput")
    # DMA-side view iterating (j, h, b, cols): outer dim 64 for 16-way fan-out.
    out_perm = out_t.ap().rearrange("b j h m -> j h b m")

    with TileContext(nc) as tc:
        with (
            tc.tile_pool(name="ins", bufs=1) as ipool,
            tc.tile_pool(name="outs", bufs=2) as opool,
        ):
            left_sb = ipool.tile([P, WC], f32, tag="left")
            rpad_sb = ipool.tile([P, TC], f32, tag="rpad")
            vrep_sb = ipool.tile([P, TC], f32, tag="vrep")
            nc.sync.dma_start(out=left_sb[:], in_=left_t[:])
            nc.sync.dma_start(out=rpad_sb[:], in_=rpad_t[:])
            nc.sync.dma_start(out=vrep_sb[:], in_=vrep_t[:])

            lv = left_sb[:].rearrange("p (w c) -> p w c", c=C)
            rv = rpad_sb[:].rearrange("p (t c) -> p t c", c=C)
            vv = vrep_sb[:].rearrange("p (t c) -> p t c", c=C)

            for j in range(DPC):
                for wi in range(0, W, WCHUNK):
                    ot = opool.tile([P, WCHUNK * 2 * C], f32, tag="ot")
                    ov = ot[:].rearrange("p (w c) -> p w c", c=2 * C)
                    t0 = wi + JPAD - j
                    nc.vector.tensor_mul(
                        out=ov[:, :, 0:C],
                        in0=lv[:, wi : wi + WCHUNK, :],
                        in1=vv[:, t0 : t0 + WCHUNK, :],
                    )
                    nc.vector.tensor_copy(
                        out=ov[:, :, C : 2 * C],
                        in_=rv[:, t0 : t0 + WCHUNK, :],
                    )
                    nc.sync.dma_start(
                        out=out_perm[j, :, :, wi * 2 * C : (wi + WCHUNK) * 2 * C],
                        in_=ot[:],
                    )
    nc.finalize()
    return nc


def get_nc():
    if "nc" not in _CACHE:
        _CACHE["nc"] = _build_nc()
    return _CACHE["nc"]


def _hb_major(x):
    """[B, H, rest...] -> [128 = (h, b) h-major, prod(rest)] contiguous."""
    return np.ascontiguousarray(x.transpose(1, 0, 2, 3)).reshape(P, -1)


def prep_inputs(left, right):
    """Build the 8 per-core input maps from full left/right."""
    left = np.ascontiguousarray(left, dtype=F32)
    right = np.ascontiguousarray(right, dtype=F32)
    left_flat = _hb_major(left)
    in_maps = []
    for k in range(N_CORES):
        d0 = DPC * k - MAX_DISP
        shift = JPAD + d0        # rpad[..., t, :] = right[..., t - shift, :]
        rpad = np.zeros((B, H, TPAD, C), F32)
        lo, hi = max(0, shift), min(TPAD, shift + W)
        if lo < hi:
            rpad[:, :, lo:hi, :] = right[:, :, lo - shift : hi - shift, :]
        vk = np.zeros(TPAD, F32)
        vk[lo:hi] = 1.0
        vrep = np.ascontiguousarray(
            np.broadcast_to(np.repeat(vk, C), (P, TC)), dtype=F32
        )
        in_maps.append(
            {"left_flat": left_flat, "rpad": _hb_major(rpad), "vrep": vrep}
        )
    return in_maps


def run(left, right, **kwargs):
    """Run the SPMD kernel; returns (full_output, BassKernelResults)."""
    from concourse.bass_utils import run_bass_kernel_spmd

    nc = get_nc()
    in_maps = prep_inputs(left, right)
    res = run_bass_kernel_spmd(nc, in_maps, core_ids=list(range(N_CORES)), **kwargs)
    full = np.concatenate(
        [r["out"].reshape(B, DPC, H, W, 2 * C) for r in res.results], axis=1
    )
    return full, res


def kernel(left, right):
    full, _ = run(left, right)
    return full
